# revision 1
# baseline (speedup 1.0000x reference)
"""Trainium2 Bass kernel for nn_AttentionModuleEx1 (LKA-style attention module).

Per-sample computation (512 ch, 64x64 spatial):
  attn = dw5x5(x) + b0
  a_i  = dwH(dwW(attn)) dilated separable branches (k=7,11,21, dil=3)
  s    = attn + a0 + a1 + a2
  y    = (W3 @ s + b3) * x        (1x1 pointwise conv over channels)

Sharding: pure data-parallel — batch 8 -> 1 sample per NeuronCore.

Implementation (fp16 compute, fp32 PSUM accumulation):
  - channels on partitions (4 blocks of 128), spatial on the free dim;
    zero-padded SBUF canvases make every conv tap a shifted-AP read.
    Canvases are double-buffered so consecutive channel blocks overlap
    across engines.
  - PE lane: 5x5, W11, W21, H7, H11 conv taps run as diagonal matmuls
    (lhsT = diag(w_tap), host-built) accumulating in PSUM; ScalarE copies
    PSUM->SBUF with the conv bias fused (Identity activation).
  - DVE lane: W7 and H21 taps as fused MACs (scalar_tensor_tensor with
    per-partition scalar weights).
  - pointwise conv: PE matmuls, lhsT = W3^T tiles (transposed on host);
    bias-add + multiply-by-x fused into one scalar_tensor_tensor per chunk.
"""

import sys

for p in ("/opt/trn_rl_repo", "/opt/pypackages"):
    if p not in sys.path:
        sys.path.insert(0, p)

import os

import numpy as np

C, H, W = 512, 64, 64
# per-block taps of the k=21 H-conv moved from the DVE lane to the PE lane
H21SPLIT = [int(v) for v in os.environ.get("H21SPLIT", "0,0,0,2").split(",")]
H21_PE = max(H21SPLIT)
# taps of the k=21 H-conv handed to the GPSIMD lane (ACT scale + Pool add),
# taken right after the PE head; DVE gets the remaining tail
H21POOL = [int(v) for v in os.environ.get("H21POOL", "0,0,0,0").split(",")]
# per-block PE taps of the k=11 W-conv (DVE takes the tail taps)
W11SPLIT = [int(v) for v in os.environ.get("W11SPLIT", "9,9,9,11").split(",")]
W7_PE = int(os.environ.get("W7_PE", "0"))
NBLK = 4  # channel blocks of 128
P = 128

_NC = None


def _build_nc():
    import concourse.bass as bass  # noqa: F401
    import concourse.bacc as bacc
    import concourse.mybir as mybir
    from concourse.tile import TileContext

    f32 = mybir.dt.float32
    f16 = mybir.dt.float16
    A = mybir.AluOpType
    AF = mybir.ActivationFunctionType

    nc = bacc.Bacc(None, target_bir_lowering=False)

    x_d = nc.dram_tensor("x", [C, H, W], f16, kind="ExternalInput")
    b0_d = nc.dram_tensor("b0", [C, 1], f32, kind="ExternalInput")
    w0_1_d = nc.dram_tensor("w0_1", [C, 7], f32, kind="ExternalInput")
    w1_1_d = nc.dram_tensor("w1_1", [C, 11], f32, kind="ExternalInput")
    w2_2_d = nc.dram_tensor("w2_2", [C, 21], f32, kind="ExternalInput")
    bsumh_d = nc.dram_tensor("bsumh", [C, 1], f32, kind="ExternalInput")
    # host-built diagonal weight stacks for the PE lane, f16:
    wd5_d = nc.dram_tensor("wd5", [NBLK, 25, P, P], f16, kind="ExternalInput")
    wd11w_d = nc.dram_tensor("wd11w", [NBLK, 11, P, P], f16, kind="ExternalInput")
    wd21w_d = nc.dram_tensor("wd21w", [NBLK, 21, P, P], f16, kind="ExternalInput")
    wd7h_d = nc.dram_tensor("wd7h", [NBLK, 7, P, P], f16, kind="ExternalInput")
    wd11h_d = nc.dram_tensor("wd11h", [NBLK, 11, P, P], f16, kind="ExternalInput")
    wd21h_d = (nc.dram_tensor("wd21h", [NBLK, 21, P, P], f16,
                              kind="ExternalInput") if H21_PE else None)
    wd7w_d = (nc.dram_tensor("wd7w", [NBLK, 7, P, P], f16,
                             kind="ExternalInput") if W7_PE else None)
    w3_d = nc.dram_tensor("w3", [C, C], f16, kind="ExternalInput")  # W3^T (host)
    b3_d = nc.dram_tensor("b3", [C, 1], f32, kind="ExternalInput")
    out_d = nc.dram_tensor("out", [C, H, W], f16, kind="ExternalOutput")

    with TileContext(nc) as tc:
        with tc.tile_pool(name="main", bufs=1) as MP, \
             tc.tile_pool(name="canv", bufs=2) as CP, \
             tc.tile_pool(name="psum", bufs=2, space="PSUM") as PP, \
             tc.tile_pool(name="diag", bufs=16) as DP, \
             tc.tile_pool(name="stage", bufs=8) as SP:

            accs = [MP.tile([P, H * W], f16, tag=f"acc{b}", name=f"acc{b}")
                    for b in range(NBLK)]

            # SBUF-resident weights
            w3ts = []  # [k][m] lhsT tiles [cin 128, cout 128]
            for kk in range(NBLK):
                row = []
                for m in range(NBLK):
                    t = MP.tile([P, P], f16, tag=f"w3t{kk}{m}", name=f"w3t{kk}{m}")
                    nc.sync.dma_start(
                        t[:, :], w3_d[kk * P:(kk + 1) * P, m * P:(m + 1) * P])
                    row.append(t)
                w3ts.append(row)

            wtiles = {}
            percol = (("b0", b0_d, 1), ("w0_1", w0_1_d, 7),
                      ("w1_1", w1_1_d, 11), ("w2_2", w2_2_d, 21),
                      ("bsumh", bsumh_d, 1), ("b3", b3_d, 1))
            for b in range(NBLK):
                sl = slice(b * P, (b + 1) * P)
                for nm, dd, k in percol:
                    t = MP.tile([P, k], f32, tag=f"{nm}_{b}", name=f"{nm}_{b}")
                    nc.sync.dma_start(t[:, :], dd[sl, :])
                    wtiles[(nm, b)] = t

            def pe_conv(b, ps_list, groups, dests, bias_ap):
                """groups: list of (diag_dram, ktaps, rview) accumulated into
                one PSUM group per 2048-chunk; dests: list of per-chunk dest
                AP callables (each gets two 16-row ACT copies, bias fused)."""
                for c in range(2):           # two 2048 chunks (32 rows each)
                    ps = PP.tile([P, 2048], f32, tag="ps", name="ps")
                    first = True
                    ngrp = len(groups)
                    for gi, (dd, kt, rv) in enumerate(groups):
                        for t in range(kt):
                            dt_t = DP.tile([P, P], f16, tag="diag", name="diag")
                            nc.sync.dma_start(dt_t[:, :], dd[b, t])
                            last = (gi == ngrp - 1) and (t == kt - 1)
                            for j in range(4):   # four N=512 matmuls per tap
                                r0 = 32 * c + 8 * j
                                nc.tensor.matmul(
                                    ps[:, 512 * j:512 * (j + 1)],
                                    dt_t[:, :], rv(t, r0, r0 + 8),
                                    start=first, stop=last)
                            first = False
                    ps3 = ps.rearrange("p (a b) -> p a b", a=32)
                    for dest in dests:
                        for half in range(2):
                            nc.scalar.activation(
                                dest(c, half), ps3[:, 16 * half:16 * half + 16, :],
                                AF.Identity, bias=bias_ap, scale=1.0)
                    ps_list.append(ps)

            for b in range(NBLK):
                sl = slice(b * P, (b + 1) * P)
                # double-buffered canvases; zero the pads on first use of
                # each of the two pool slots, interiors are always rewritten
                xcan = CP.tile([P, 68, 68], f16, tag="xcan", name="xcan")
                attncan = CP.tile([P, 64, 124], f16, tag="attncan", name="attncan")
                z7 = CP.tile([P, 82, 64], f16, tag="z7", name="z7")
                z11 = CP.tile([P, 94, 64], f16, tag="z11", name="z11")
                z21 = CP.tile([P, 124, 64], f16, tag="z21", name="z21")
                if b < 2:
                    nc.gpsimd.memset(xcan[:, :, :], 0.0)
                    nc.gpsimd.memset(attncan[:, :, :], 0.0)
                    nc.gpsimd.memset(z7[:, :, :], 0.0)
                    nc.gpsimd.memset(z11[:, :, :], 0.0)
                    nc.gpsimd.memset(z21[:, :, :], 0.0)
                nc.gpsimd.dma_start(xcan[:, 2:66, 2:66], x_d[sl, :, :])
                attn_int = attncan[:, :, 30:94]
                acc3 = accs[b].rearrange("p (a b) -> p a b", a=H)

                # ---- 5x5 depthwise on PE (25 diag matmuls, pad 2);
                #      writes attn interior AND acc (bias b0 fused) ----
                def rv5(t, r0, r1):
                    dh, dw = t // 5, t % 5
                    return xcan[:, dh + r0:dh + r1, dw:dw + 64]

                pe_conv(b, [], [(wd5_d, 25, rv5)],
                        [lambda c, h2: attn_int[:, 32 * c + 16 * h2:
                                                32 * c + 16 * h2 + 16, :]],
                        wtiles[("b0", b)][:, 0:1])

                # ---- W-convs ----
                def rvw(pad):
                    def rv(t, r0, r1):
                        col0 = 30 + 3 * t - pad
                        return attncan[:, r0:r1, col0:col0 + 64]
                    return rv

                def destz(zc, zpad):
                    def dest(c, h2):
                        r = zpad + 32 * c + 16 * h2
                        return zc[:, r:r + 16, :]
                    return dest

                # k=21 on PE (into z21 rows 30..94)
                pe_conv(b, [], [(wd21w_d, 21, rvw(30))],
                        [destz(z21, 30)], 0.0)
                # k=11: first W11SPLIT[b] taps on PE, tail on DVE
                w11_pe = W11SPLIT[b]
                z11i = z11[:, 15:79, :]
                if w11_pe:
                    pe_conv(b, [], [(wd11w_d, w11_pe, rvw(15))],
                            [destz(z11, 15)], 0.0)
                w11t = wtiles[("w1_1", b)]
                for t in range(w11_pe, 11):
                    col0 = 30 + 3 * t - 15
                    av = attncan[:, :, col0:col0 + 64]
                    if t == 0:
                        nc.vector.tensor_scalar_mul(z11i, av, w11t[:, 0:1])
                    else:
                        nc.vector.scalar_tensor_tensor(
                            z11i, av, w11t[:, t:t + 1], z11i,
                            op0=A.mult, op1=A.add)

                # k=7: first W7_PE taps on PE, rest on DVE (STT chain), pad 9
                z7i = z7[:, 9:73, :]
                w1t = wtiles[("w0_1", b)]
                if W7_PE:
                    pe_conv(b, [], [(wd7w_d, W7_PE, rvw(9))],
                            [destz(z7, 9)], 0.0)
                for c in range(2):
                    z7c = z7[:, 9 + 32 * c:9 + 32 * c + 32, :]
                    for t in range(W7_PE, 7):
                        col0 = 30 + 3 * t - 9
                        av3 = attncan[:, 32 * c:32 * c + 32, col0:col0 + 64]
                        if t == 0:
                            nc.vector.tensor_scalar_mul(z7c, av3, w1t[:, 0:1])
                        else:
                            tmp = SP.tile([P, 32, 64], f16, tag="ptmp", bufs=2,
                                          name="ptmp")
                            nc.vector.tensor_scalar_mul(
                                tmp[:, :, :], av3, w1t[:, t:t + 1])
                            nc.vector.tensor_tensor(
                                z7c, tmp[:, :, :], z7c, op=A.add)

                # ---- H-convs ----
                # k=7 + k=11 (+ first H21_PE taps of k=21) fused on PE into
                # one PSUM accumulation
                def rvh(zc):
                    def rv(t, r0, r1):
                        row0 = 3 * t  # zpad + 3t - pad = 3t for all branches
                        return zc[:, row0 + r0:row0 + r1, :]
                    return rv

                h21_pe = H21SPLIT[b]
                hgroups = [(wd7h_d, 7, rvh(z7)), (wd11h_d, 11, rvh(z11))]
                if h21_pe:
                    hgroups.append((wd21h_d, h21_pe, rvh(z21)))
                hsum = SP.tile([P, H * W], f16, tag="hsum", bufs=2, name="hsum")
                hsum3 = hsum.rearrange("p (a b) -> p a b", a=H)
                pe_conv(b, [], hgroups,
                        [lambda c, h2: hsum3[:, 32 * c + 16 * h2:
                                             32 * c + 16 * h2 + 16, :]],
                        wtiles[("bsumh", b)][:, 0:1])

                # k=21 H-conv: next H21POOL[b] taps on the GPSIMD lane
                # (ACT per-partition scale into tmp chunks, Pool adds into a
                # private accumulator), remaining tail on DVE (STT into acc)
                w2t = wtiles[("w2_2", b)]
                h21_pool = H21POOL[b]
                pool_accs = []
                if h21_pool:
                    for c in range(2):
                        pacc = SP.tile([P, 2048], f16, tag="pacc", bufs=2,
                                       name="pacc")
                        for ti in range(h21_pool):
                            t = h21_pe + ti
                            zv3 = z21[:, 3 * t + 32 * c:3 * t + 32 * c + 32, :]
                            if ti == 0:
                                nc.scalar.activation(
                                    pacc.rearrange("p (a b) -> p a b", a=32),
                                    zv3, AF.Identity, bias=0.0,
                                    scale=w2t[:, t:t + 1])
                            else:
                                tmp = SP.tile([P, 2048], f16, tag="ptmp",
                                              bufs=2, name="ptmp")
                                nc.scalar.activation(
                                    tmp.rearrange("p (a b) -> p a b", a=32),
                                    zv3, AF.Identity, bias=0.0,
                                    scale=w2t[:, t:t + 1])
                                nc.gpsimd.tensor_tensor(
                                    pacc[:, :], tmp[:, :], pacc[:, :], op=A.add)
                        pool_accs.append(pacc)
                t0 = h21_pe + h21_pool
                for c in range(2):
                    av3 = acc3[:, 32 * c:32 * c + 32, :]
                    for t in range(t0, 21):
                        zv3 = z21[:, 3 * t + 32 * c:3 * t + 32 * c + 32, :]
                        tmp = SP.tile([P, 32, 64], f16, tag="ptmp", bufs=2,
                                      name="ptmp")
                        nc.vector.tensor_scalar_mul(
                            tmp[:, :, :], zv3, w2t[:, t:t + 1])
                        nc.vector.tensor_tensor(
                            av3, tmp[:, :, :],
                            attn_int[:, 32 * c:32 * c + 32, :] if t == t0
                            else av3, op=A.add)
                if h21_pe + h21_pool == 21 and h21_pool:
                    # no DVE tap ran: fold attn into the pool accumulator
                    for c in range(2):
                        nc.gpsimd.tensor_tensor(
                            pool_accs[c].rearrange("p (a b) -> p a b", a=32),
                            attn_int[:, 32 * c:32 * c + 32, :],
                            pool_accs[c].rearrange("p (a b) -> p a b", a=32),
                            op=A.add)

                # merge the GPSIMD-lane accumulator into acc (DVE)
                for c, pacc in enumerate(pool_accs):
                    av = accs[b][:, 2048 * c:2048 * (c + 1)]
                    nc.vector.scalar_tensor_tensor(
                        av, pacc[:, :], 1.0, av, op0=A.mult, op1=A.add)

                # combine: acc += (H7 + H11 [+H21 head] + all folded biases);
                # when the whole H21 ran on PE, acc = attn + hsum instead
                for c in range(2):
                    av3 = acc3[:, 32 * c:32 * c + 32, :]
                    in1 = (av3 if h21_pe < 21 else
                           attn_int[:, 32 * c:32 * c + 32, :])
                    nc.vector.tensor_tensor(
                        av3, hsum3[:, 32 * c:32 * c + 32, :], in1, op=A.add)

            # ---- pointwise 1x1 conv + bias + multiply-by-x ----
            NCH = 8  # 512-column chunks of the 4096 spatial dim
            for m in range(NBLK):
                sl = slice(m * P, (m + 1) * P)
                xcan = CP.tile([P, 68, 68], f16, tag="xcan", name="xcan")
                nc.gpsimd.dma_start(xcan[:, 2:66, 2:66], x_d[sl, :, :])
                for nch in range(NCH):
                    ps = PP.tile([P, 512], f32, tag="ps", name="ps")
                    for kk in range(NBLK):
                        nc.tensor.matmul(
                            ps[:, :], w3ts[kk][m][:, :],
                            accs[kk][:, nch * 512:(nch + 1) * 512],
                            start=(kk == 0), stop=(kk == NBLK - 1))
                    yb = SP.tile([P, 8, 64], f16, tag="yb", name="yb")
                    ps3 = ps.rearrange("p (a b) -> p a b", a=8)
                    nc.scalar.activation(yb[:, :, :], ps3, AF.Identity,
                                         bias=wtiles[("b3", m)][:, 0:1],
                                         scale=1.0)
                    ost = SP.tile([P, 8, 64], f16, tag="ost", name="ost")
                    xv = xcan[:, 2 + 8 * nch:2 + 8 * nch + 8, 2:66]
                    nc.vector.tensor_tensor(
                        ost[:, :, :], yb[:, :, :], xv, op=A.mult)
                    nc.sync.dma_start(
                        out_d[sl, 8 * nch:8 * nch + 8, :], ost[:, :, :])

    if not nc.is_finalized():
        nc.finalize()
    return nc


def _get_nc():
    global _NC
    if _NC is None:
        _NC = _build_nc()
    return _NC


def _diag_stack(w):
    """w: (C, k) f32 -> (NBLK, k, P, P) f16 diagonal stacks."""
    k = w.shape[1]
    d = np.zeros((NBLK, k, P, P), np.float16)
    idx = np.arange(P)
    wb = w.reshape(NBLK, P, k).transpose(0, 2, 1).astype(np.float16)  # (4,k,128)
    d[:, :, idx, idx] = wb
    return np.ascontiguousarray(d)


def _prep_inputs(inputs):
    f = lambda a, shp: np.ascontiguousarray(
        np.asarray(a, dtype=np.float32).reshape(shp))
    g = lambda nm, k: f(inputs[nm], (C, k))
    com = {
        "b0": f(inputs["b0"], (C, 1)),
        "w0_1": g("w0_1", 7),
        "w1_1": g("w1_1", 11),
        "w2_2": g("w2_2", 21),
        "bsumh": f(np.asarray(inputs["b0_2"], np.float32)
                   + np.asarray(inputs["b1_2"], np.float32)
                   + np.asarray(inputs["b2_2"], np.float32)
                   + g("w0_2", 7).sum(1) * np.asarray(inputs["b0_1"], np.float32).reshape(C)
                   + g("w1_2", 11).sum(1) * np.asarray(inputs["b1_1"], np.float32).reshape(C)
                   + g("w2_2", 21).sum(1) * np.asarray(inputs["b2_1"], np.float32).reshape(C),
                   (C, 1)),
        "wd5": _diag_stack(g("w0", 25)),
        "wd11w": _diag_stack(g("w1_1", 11)),
        "wd21w": _diag_stack(g("w2_1", 21)),
        "wd7h": _diag_stack(g("w0_2", 7)),
        "wd11h": _diag_stack(g("w1_2", 11)),
        **({"wd21h": _diag_stack(g("w2_2", 21))} if H21_PE else {}),
        **({"wd7w": _diag_stack(g("w0_1", 7))} if W7_PE else {}),
        "w3": np.ascontiguousarray(
            np.asarray(inputs["w3"], np.float32).reshape(C, C).T
            .astype(np.float16)),
        "b3": f(inputs["b3"], (C, 1)),
    }
    x = np.asarray(inputs["x"], np.float32).astype(np.float16)
    return [dict(com, x=np.ascontiguousarray(x[i])) for i in range(x.shape[0])]


def run(inputs, trace=False):
    from concourse.bass_utils import run_bass_kernel_spmd
    nc = _get_nc()
    in_maps = _prep_inputs(inputs)
    res = run_bass_kernel_spmd(nc, in_maps, core_ids=list(range(len(in_maps))),
                               trace=trace)
    out = np.stack([r["out"] for r in res.results], axis=0).astype(np.float32)
    return out, res


def kernel(**inputs):
    out, _ = run(inputs, trace=False)
    return out



# revision 8
# speedup vs baseline: 1.4528x; 1.4528x over previous
"""Trainium2 Bass kernel for nn_AttentionModuleEx1 (LKA-style attention module).

Per-sample computation (512 ch, 64x64 spatial):
  attn = dw5x5(x) + b0
  a_i  = dwH(dwW(attn)) dilated separable branches (k=7,11,21, dil=3)
  s    = attn + a0 + a1 + a2          (+ folded branch biases)
  y    = (W3 @ s + b3) * x            (1x1 pointwise conv over channels)

Sharding: pure data-parallel - batch 8 -> 1 sample per NeuronCore.

Implementation:
  - channels on partitions (4 blocks of 128), spatial on the free dim.
  - 5x5 depthwise: fp16 diagonal matmuls on PE (SBUF-resident diag stacks,
    one batched DMA per block) + a tunable number of taps offloaded to the
    DVE (tensor_scalar) and ACT (activation-scale) lanes, whose partial
    products are folded into the accumulator with SWDGE accumulate-DMAs.
  - dilated separable branches: fp8e4 DoubleRow diagonal matmuls (two taps
    per PE instruction at 0.5 cycles/row). Branch weights are scaled x16
    into fp8's normal range; the H-conv eviction rescales by 1/256 exactly.
    Branch canvases (attn, z7/z11/z21) are fp8.
  - H-convs of all three branches accumulate in one PSUM group; eviction
    fuses the x(1/256) rescale and the folded branch-bias vector.
  - pointwise conv: fp16 PE matmuls (lhsT = W3^T row-tiles); bias via ACT
    eviction, multiply-by-x on DVE, one batched output DMA per block.
  - emission is stage-major (all 5x5 stages, then all branch stages, then
    pointwise) so the per-block accumulate-DMA chains overlap PE work on
    other blocks.
"""

import os
import sys

for p in ("/opt/trn_rl_repo", "/opt/pypackages"):
    if p not in sys.path:
        sys.path.insert(0, p)

import numpy as np

C, H, W = 512, 64, 64
NBLK = 4
P = 128
HW = H * W

# 5x5 tap split per block: PE, DVE, ACT (must sum to 25)
K5 = [int(v) for v in os.environ.get("K5", "8,13,4").split(",")]
assert sum(K5) == 25
NO_BRANCH = os.environ.get("NO_BRANCH", "0") == "1"
NO_LANES = os.environ.get("NO_LANES", "0") == "1"
if NO_LANES:
    K5 = [25, 0, 0]
BR_SCALE = 16.0  # branch weights are stored x16 in fp8

# (branch, taps, pairs, W-pad, z rows incl. OOB guard, z lead pad)
WBR = (("7", 7, 4, 9, 85, 9), ("11", 11, 6, 15, 97, 15),
       ("21", 21, 11, 30, 127, 30))

_NC = None


def _build_nc():
    import concourse.bass as bass  # noqa: F401
    import concourse.bacc as bacc
    import concourse.mybir as mybir
    from concourse.bass import AP
    from concourse.tile import TileContext

    f32 = mybir.dt.float32
    f16 = mybir.dt.float16
    f8 = mybir.dt.float8e4
    A = mybir.AluOpType
    AF = mybir.ActivationFunctionType
    DR = mybir.MatmulPerfMode.DoubleRow

    nc = bacc.Bacc(None, target_bir_lowering=False)

    xc_d = nc.dram_tensor("xcanv", [C, 68, 68], f16, kind="ExternalInput")
    b0_d = nc.dram_tensor("b0", [C, 1], f32, kind="ExternalInput")
    w0sc_d = nc.dram_tensor("w0sc", [C, 25], f32, kind="ExternalInput")
    bsumh_d = nc.dram_tensor("bsumh", [C, 1], f32, kind="ExternalInput")
    wd5_d = nc.dram_tensor("wd5", [NBLK, P, 25, P], f16, kind="ExternalInput")
    brd = {}
    for nm, k, npair, wpad, zrows, zpad in WBR:
        brd["w" + nm] = nc.dram_tensor(
            "wdw" + nm, [NBLK, P, npair, 2, P], f8, kind="ExternalInput")
        brd["h" + nm] = nc.dram_tensor(
            "wdh" + nm, [NBLK, P, npair, 2, P], f8, kind="ExternalInput")
    w3_d = nc.dram_tensor("w3", [C, C], f16, kind="ExternalInput")  # W3^T
    b3_d = nc.dram_tensor("b3", [C, 1], f32, kind="ExternalInput")
    out_d = nc.dram_tensor("out", [C, H, W], f16, kind="ExternalOutput")

    with TileContext(nc) as tc:
        with tc.tile_pool(name="main", bufs=1) as MP, \
             tc.tile_pool(name="canv", bufs=2) as CP, \
             tc.tile_pool(name="attn", bufs=4) as APool, \
             tc.tile_pool(name="psum", bufs=2, space="PSUM") as PP, \
             tc.tile_pool(name="diag", bufs=2) as DP, \
             tc.tile_pool(name="bdiag", bufs=1) as BDP, \
             tc.tile_pool(name="stage", bufs=2) as SP:

            accs = [MP.tile([P, HW], f16, tag=f"acc{b}", name=f"acc{b}")
                    for b in range(NBLK)]
            attn8s = [APool.tile([P, 64, 128], f8, tag="attn8",
                                 name=f"attn8_{b}") for b in range(NBLK)]

            # SBUF-resident pointwise weights: 4 row-bands of W3^T
            w3rows = []
            for kk in range(NBLK):
                t = MP.tile([P, C], f16, tag=f"w3r{kk}", name=f"w3r{kk}")
                nc.sync.dma_start(t[:, :], w3_d[kk * P:(kk + 1) * P, :])
                w3rows.append(t)

            wtiles = {}
            for b in range(NBLK):
                sl = slice(b * P, (b + 1) * P)
                for nm, dd, k in (("b0", b0_d, 1), ("bsumh", bsumh_d, 1),
                                  ("b3", b3_d, 1), ("w0sc", w0sc_d, 25)):
                    t = MP.tile([P, k], f32, tag=f"{nm}_{b}", name=f"{nm}_{b}")
                    nc.sync.dma_start(t[:, :], dd[sl, :])
                    wtiles[(nm, b)] = t

            # zero the OOB-guard pad regions of the fp8 canvases (both pool
            # slots; interiors are always rewritten)
            def zero_pads(a8, z_tiles):
                nc.gpsimd.memset(a8[:, :, 0:30], 0.0)
                nc.gpsimd.memset(a8[:, :, 94:128], 0.0)
                for zt, (nm, k, npair, wpad, zrows, zpad) in zip(z_tiles, WBR):
                    nc.gpsimd.memset(zt[:, 0:zpad, :], 0.0)
                    nc.gpsimd.memset(zt[:, zpad + 64:zrows, :], 0.0)

            # ---------------- stage A: 5x5 depthwise per block -------------
            xcans = {}
            for b in range(NBLK):
                sl = slice(b * P, (b + 1) * P)
                xcan = CP.tile([P, 68, 68], f16, tag="xcan", name="xcan")
                xcans[b] = xcan
                nc.sync.dma_start(xcan[:, :, :], xc_d[sl, :, :])
                d5 = DP.tile([P, 25, P], f16, tag="d5", name="d5")
                nc.sync.dma_start(d5[:, :, :], wd5_d[b])

                acc3 = accs[b].rearrange("p (a b) -> p a b", a=H)

                def rv5(t, r0, r1):
                    dh, dw = t // 5, t % 5
                    return xcan[:, dh + r0:dh + r1, dw:dw + 64]

                n_pe, n_dve, n_act = K5
                for c in range(2):
                    ps = PP.tile([P, 2048], f32, tag="ps", name="ps")
                    for i in range(n_pe):
                        for j in range(4):
                            r0 = 32 * c + 8 * j
                            nc.tensor.matmul(
                                ps[:, 512 * j:512 * (j + 1)], d5[:, i, :],
                                rv5(i, r0, r0 + 8),
                                start=(i == 0), stop=(i == n_pe - 1))
                    ps3 = ps.rearrange("p (a b) -> p a b", a=32)
                    for h2 in range(2):
                        nc.scalar.activation(
                            acc3[:, 32 * c + 16 * h2:32 * c + 16 * h2 + 16, :],
                            ps3[:, 16 * h2:16 * h2 + 16, :],
                            AF.Identity, bias=wtiles[("b0", b)][:, 0:1],
                            scale=1.0)

                w0sc = wtiles[("w0sc", b)]
                for i in range(n_dve):
                    t = n_pe + i
                    tmp = SP.tile([P, HW], f16, tag="ttmp", bufs=2,
                                  name="ttmp")
                    nc.vector.tensor_scalar_mul(
                        tmp.rearrange("p (a b) -> p a b", a=H),
                        rv5(t, 0, 64), w0sc[:, t:t + 1])
                    nc.vector.tensor_tensor(
                        accs[b][:, :], tmp[:, :], accs[b][:, :], op=A.add)
                if n_act:
                    # ACT-lane taps chain through a Pool-side accumulator
                    pacc = SP.tile([P, HW], f16, tag="pacc", name="pacc")
                    for i in range(n_act):
                        t = n_pe + n_dve + i
                        if i == 0:
                            nc.scalar.activation(
                                pacc.rearrange("p (a b) -> p a b", a=H),
                                rv5(t, 0, 64), AF.Identity, bias=0.0,
                                scale=w0sc[:, t:t + 1])
                        else:
                            tmp = SP.tile([P, HW], f16, tag="ttmp", bufs=2,
                                          name="ttmp")
                            nc.scalar.activation(
                                tmp.rearrange("p (a b) -> p a b", a=H),
                                rv5(t, 0, 64), AF.Identity, bias=0.0,
                                scale=w0sc[:, t:t + 1])
                            nc.gpsimd.tensor_tensor(
                                pacc[:, :], tmp[:, :], pacc[:, :], op=A.add)
                    nc.vector.tensor_tensor(
                        accs[b][:, :], pacc[:, :], accs[b][:, :], op=A.add)

                # fp8 mirror of attn for the branch convs
                nc.vector.tensor_scalar_mul(
                    attn8s[b][:, :, 30:94], acc3, 1.0)

            # --------------- stage B: dilated branches per block -----------
            for b in range(NBLK if not NO_BRANCH else 0):
                attn8 = attn8s[b]
                zts = []
                for nm, k, npair, wpad, zrows, zpad in WBR:
                    zts.append(CP.tile([P, zrows, 64], f8, tag=f"z{nm}",
                                       name=f"z{nm}"))
                if b < 2:
                    zero_pads(attn8, zts)
                else:
                    nc.gpsimd.memset(attn8[:, :, 0:30], 0.0)
                    nc.gpsimd.memset(attn8[:, :, 94:128], 0.0)
                dsw, dsh = [], []
                for nm, k, npair, wpad, zrows, zpad in WBR:
                    dw_t = BDP.tile([P, npair, 2, P], f8, tag=f"dw{nm}",
                                    name=f"dw{nm}")
                    nc.sync.dma_start(dw_t[:, :, :, :], brd["w" + nm][b])
                    dh_t = BDP.tile([P, npair, 2, P], f8, tag=f"dh{nm}",
                                    name=f"dh{nm}")
                    nc.sync.dma_start(dh_t[:, :, :, :], brd["h" + nm][b])
                    dsw.append(dw_t)
                    dsh.append(dh_t)

                # W-convs: attn8 -> z (fp8 DoubleRow pairs, shift = 3 cols)
                for (nm, k, npair, wpad, zrows, zpad), dw_t, zt in \
                        zip(WBR, dsw, zts):
                    for c in range(2):
                        ps = PP.tile([P, 2048], f32, tag="ps", name="ps")
                        for jp in range(npair):
                            col0 = 30 + 6 * jp - wpad
                            for j in range(4):
                                r0 = 32 * c + 8 * j
                                base = attn8[:, r0:r0 + 8, col0:col0 + 64]
                                rhs = AP(base.tensor, base.offset,
                                         [base.ap[0], [3, 2], [128, 8],
                                          [1, 64]])
                                nc.tensor.matmul(
                                    ps[:, 512 * j:512 * (j + 1)],
                                    dw_t[:, jp, :, :], rhs,
                                    start=(jp == 0), stop=(jp == npair - 1),
                                    perf_mode=DR)
                        ps3 = ps.rearrange("p (a b) -> p a b", a=32)
                        for h2 in range(2):
                            r = zpad + 32 * c + 16 * h2
                            nc.scalar.activation(
                                zt[:, r:r + 16, :],
                                ps3[:, 16 * h2:16 * h2 + 16, :],
                                AF.Identity, bias=0.0, scale=1.0)

                # H-convs: all branches -> one PSUM group (shift = 3 rows)
                hsum = SP.tile([P, HW], f16, tag="hsum", bufs=1, name="hsum")
                hsum3 = hsum.rearrange("p (a b) -> p a b", a=H)
                ngrp = len(WBR)
                for c in range(2):
                    ps = PP.tile([P, 2048], f32, tag="ps", name="ps")
                    for gi, ((nm, k, npair, wpad, zrows, zpad), dh_t, zt) in \
                            enumerate(zip(WBR, dsh, zts)):
                        for jp in range(npair):
                            row0 = 6 * jp
                            for j in range(4):
                                r0 = 32 * c + 8 * j
                                base = zt[:, row0 + r0:row0 + r0 + 8, :]
                                rhs = AP(base.tensor, base.offset,
                                         [base.ap[0], [192, 2], [64, 8],
                                          [1, 64]])
                                nc.tensor.matmul(
                                    ps[:, 512 * j:512 * (j + 1)],
                                    dh_t[:, jp, :, :], rhs,
                                    start=(gi == 0 and jp == 0),
                                    stop=(gi == ngrp - 1 and
                                          jp == npair - 1),
                                    perf_mode=DR)
                    ps3 = ps.rearrange("p (a b) -> p a b", a=32)
                    for h2 in range(2):
                        nc.scalar.activation(
                            hsum3[:, 32 * c + 16 * h2:
                                  32 * c + 16 * h2 + 16, :],
                            ps3[:, 16 * h2:16 * h2 + 16, :],
                            AF.Identity, bias=wtiles[("bsumh", b)][:, 0:1],
                            scale=1.0 / (BR_SCALE * BR_SCALE))

                # s = attn (+ lane taps) + hsum
                for c in range(2):
                    av = accs[b][:, 2048 * c:2048 * (c + 1)]
                    nc.vector.tensor_tensor(
                        av, hsum[:, 2048 * c:2048 * (c + 1)], av, op=A.add)

            # ---- pointwise 1x1 conv + bias + multiply-by-x ----
            for m in range(NBLK if os.environ.get("NO_PW", "0") != "1" else 1):
                sl = slice(m * P, (m + 1) * P)
                xcan = CP.tile([P, 68, 68], f16, tag="xcan", name="xcan")
                nc.sync.dma_start(xcan[:, :, :], xc_d[sl, :, :])
                outb = SP.tile([P, HW], f16, tag="outb", name="outb")
                outb3 = outb.rearrange("p (a b) -> p a b", a=H)
                for nch in range(8):
                    ps = PP.tile([P, 2048], f32, tag="ps", name="ps")
                    for kk in range(NBLK):
                        nc.tensor.matmul(
                            ps[:, 0:512], w3rows[kk][:, m * P:(m + 1) * P],
                            accs[kk][:, nch * 512:(nch + 1) * 512],
                            start=(kk == 0), stop=(kk == NBLK - 1))
                    yb = SP.tile([P, 8, 64], f16, tag="yb", name="yb")
                    nc.scalar.activation(
                        yb[:, :, :],
                        ps[:, 0:512].rearrange("p (a b) -> p a b", a=8),
                        AF.Identity, bias=wtiles[("b3", m)][:, 0:1],
                        scale=1.0)
                    xv = xcan[:, 2 + 8 * nch:2 + 8 * nch + 8, 2:66]
                    nc.vector.tensor_tensor(
                        outb3[:, 8 * nch:8 * nch + 8, :], yb[:, :, :], xv,
                        op=A.mult)
                nc.sync.dma_start(out_d[sl, :, :], outb3[:, :, :])

    if not nc.is_finalized():
        nc.finalize()
    return nc


def _get_nc():
    global _NC
    if _NC is None:
        _NC = _build_nc()
    return _NC


def _prep_inputs(inputs):
    import ml_dtypes
    f8np = ml_dtypes.float8_e4m3

    f = lambda a, shp: np.ascontiguousarray(
        np.asarray(a, dtype=np.float32).reshape(shp))
    g = lambda nm, k: f(inputs[nm], (C, k))

    w0 = g("w0", 25)
    wd5 = np.zeros((NBLK, P, 25, P), np.float16)
    idx = np.arange(P)
    for b in range(NBLK):
        wd5[b, idx, :, idx] = w0[b * P:(b + 1) * P, :].astype(np.float16)

    def pair_stack(w, npair):
        k = w.shape[1]
        d = np.zeros((NBLK, P, npair, 2, P), np.float32)
        for b in range(NBLK):
            wb = w[b * P:(b + 1) * P, :] * BR_SCALE
            for jp in range(npair):
                for i in range(2):
                    t = 2 * jp + i
                    if t < k:
                        d[b, idx, jp, i, idx] = wb[:, t]
        return np.ascontiguousarray(d.astype(f8np))

    com = {
        "b0": f(inputs["b0"], (C, 1)),
        "w0sc": w0,
        "bsumh": f(np.asarray(inputs["b0_2"], np.float32)
                   + np.asarray(inputs["b1_2"], np.float32)
                   + np.asarray(inputs["b2_2"], np.float32)
                   + g("w0_2", 7).sum(1) * np.asarray(inputs["b0_1"], np.float32).reshape(C)
                   + g("w1_2", 11).sum(1) * np.asarray(inputs["b1_1"], np.float32).reshape(C)
                   + g("w2_2", 21).sum(1) * np.asarray(inputs["b2_1"], np.float32).reshape(C),
                   (C, 1)),
        "wd5": np.ascontiguousarray(wd5),
        "wdw7": pair_stack(g("w0_1", 7), 4),
        "wdh7": pair_stack(g("w0_2", 7), 4),
        "wdw11": pair_stack(g("w1_1", 11), 6),
        "wdh11": pair_stack(g("w1_2", 11), 6),
        "wdw21": pair_stack(g("w2_1", 21), 11),
        "wdh21": pair_stack(g("w2_2", 21), 11),
        "w3": np.ascontiguousarray(
            np.asarray(inputs["w3"], np.float32).reshape(C, C).T
            .astype(np.float16)),
        "b3": f(inputs["b3"], (C, 1)),
    }
    x = np.asarray(inputs["x"], np.float32).astype(np.float16)
    xp = np.zeros((x.shape[0], C, 68, 68), np.float16)
    xp[:, :, 2:66, 2:66] = x
    return [dict(com, xcanv=np.ascontiguousarray(xp[i]))
            for i in range(x.shape[0])]


def run(inputs, trace=False):
    from concourse.bass_utils import run_bass_kernel_spmd
    nc = _get_nc()
    in_maps = _prep_inputs(inputs)
    res = run_bass_kernel_spmd(nc, in_maps, core_ids=list(range(len(in_maps))),
                               trace=trace)
    out = np.stack([r["out"] for r in res.results], axis=0).astype(np.float32)
    return out, res


def kernel(**inputs):
    out, _ = run(inputs, trace=False)
    return out


# revision 15
# speedup vs baseline: 1.6292x; 1.1215x over previous
"""Trainium2 Bass kernel for nn_AttentionModuleEx1 (LKA-style attention module).

Per-sample computation (512 ch, 64x64 spatial):
  attn = dw5x5(x) + b0
  a_i  = dwH(dwW(attn)) dilated separable branches (k=7,11,21, dil=3)
  s    = attn + a0 + a1 + a2
  y    = (W3 @ s + b3') * x           (1x1 pointwise conv over channels;
                                       b3' folds the branch-bias vector)

Sharding: pure data-parallel - batch 8 -> 1 sample per NeuronCore.

Implementation:
  - channels on partitions (4 blocks of 128), spatial on the free dim.
  - 5x5 depthwise: fp16 diagonal matmuls on PE (SBUF-resident diag stacks,
    one batched DMA per block) + taps offloaded to the DVE (tensor_scalar +
    add) and ACT (activation-scale + Pool-add) lanes, chained into the
    accumulator at 2048-element chunks to keep dependency chains short.
  - dilated separable branches: fp8e4 DoubleRow diagonal matmuls (two taps
    per PE instruction at 0.5 cycles/row). Branch weights are scaled x16
    into fp8's normal range; branch canvases (attn, z7/z11/z21) are fp8.
    The H-conv PSUM (x256) is folded into the accumulator with one DVE
    scalar_tensor_tensor per chunk (scale 1/256, no intermediate buffer).
  - folded branch-bias: b3' = b3 + W3 @ bsumh on the host, so no bias work
    on device beyond the two Act evictions (b0, b3').
  - pointwise conv: fp16 PE matmuls (lhsT = W3^T row-tiles); bias via ACT
    eviction, multiply-by-x on DVE, one batched output DMA per block.
  - block emission is interleaved (A0 A1 B0 A2 B1 A3 B2 B3) so stage-B PE
    work overlaps the serial lane-accumulation chains of other blocks.
"""

import os
import sys

for p in ("/opt/trn_rl_repo", "/opt/pypackages"):
    if p not in sys.path:
        sys.path.insert(0, p)

import numpy as np

C, H, W = 512, 64, 64
NBLK = 4
P = 128
HW = H * W

# 5x5 tap split per block: PE, DVE, ACT (must sum to 25)
K5 = [int(v) for v in os.environ.get("K5", "9,9,7").split(",")]
assert sum(K5) == 25
BR_SCALE = 16.0  # branch weights are stored x16 in fp8

# (branch, taps, pairs, W-pad, z rows incl. OOB guard, z lead pad)
WBR = (("7", 7, 4, 9, 85, 9), ("11", 11, 6, 15, 97, 15),
       ("21", 21, 11, 30, 127, 30))

_NC = None


def _build_nc():
    import concourse.bass as bass  # noqa: F401
    import concourse.bacc as bacc
    import concourse.mybir as mybir
    from concourse.bass import AP
    from concourse.tile import TileContext

    f32 = mybir.dt.float32
    f16 = mybir.dt.float16
    f8 = mybir.dt.float8e4
    A = mybir.AluOpType
    AF = mybir.ActivationFunctionType
    DR = mybir.MatmulPerfMode.DoubleRow

    nc = bacc.Bacc(None, target_bir_lowering=False)

    xc_d = nc.dram_tensor("xcanv", [C, 68, 68], f16, kind="ExternalInput")
    b0_d = nc.dram_tensor("b0", [C, 1], f32, kind="ExternalInput")
    w0sc_d = nc.dram_tensor("w0sc", [C, 25], f32, kind="ExternalInput")
    wd5_d = nc.dram_tensor("wd5", [NBLK, P, 25, P], f16, kind="ExternalInput")
    brd = {}
    for nm, k, npair, wpad, zrows, zpad in WBR:
        brd["w" + nm] = nc.dram_tensor(
            "wdw" + nm, [NBLK, P, npair, 2, P], f8, kind="ExternalInput")
        brd["h" + nm] = nc.dram_tensor(
            "wdh" + nm, [NBLK, P, npair, 2, P], f8, kind="ExternalInput")
    w3_d = nc.dram_tensor("w3", [C, C], f16, kind="ExternalInput")  # W3^T
    b3_d = nc.dram_tensor("b3", [C, 1], f32, kind="ExternalInput")
    out_d = nc.dram_tensor("out", [C, H, W], f16, kind="ExternalOutput")

    with TileContext(nc) as tc:
        with tc.tile_pool(name="main", bufs=1) as MP, \
             tc.tile_pool(name="canv", bufs=2) as CP, \
             tc.tile_pool(name="attn", bufs=4) as APool, \
             tc.tile_pool(name="psum", bufs=2, space="PSUM") as PP, \
             tc.tile_pool(name="diag", bufs=2) as DP, \
             tc.tile_pool(name="bdiag", bufs=int(os.environ.get("BDPB", "2"))) as BDP, \
             tc.tile_pool(name="stage", bufs=2) as SP:

            accs = [MP.tile([P, HW], f16, tag=f"acc{b}", name=f"acc{b}")
                    for b in range(NBLK)]
            attn8s = [APool.tile([P, 64, 128], f8, tag="attn8",
                                 name=f"attn8_{b}") for b in range(NBLK)]

            # SBUF-resident pointwise weights: 4 row-bands of W3^T
            w3rows = []
            for kk in range(NBLK):
                t = MP.tile([P, C], f16, tag=f"w3r{kk}", name=f"w3r{kk}")
                nc.sync.dma_start(t[:, :], w3_d[kk * P:(kk + 1) * P, :])
                w3rows.append(t)

            wtiles = {}
            for b in range(NBLK):
                sl = slice(b * P, (b + 1) * P)
                for nm, dd, k in (("b0", b0_d, 1), ("b3", b3_d, 1),
                                  ("w0sc", w0sc_d, 25)):
                    t = MP.tile([P, k], f32, tag=f"{nm}_{b}", name=f"{nm}_{b}")
                    nc.sync.dma_start(t[:, :], dd[sl, :])
                    wtiles[(nm, b)] = t

            # ---------------- stage A: 5x5 depthwise -----------------------
            def stage_a(b):
                sl = slice(b * P, (b + 1) * P)
                xcan = CP.tile([P, 68, 68], f16, tag="xcan", name="xcan")
                nc.sync.dma_start(xcan[:, :, :], xc_d[sl, :, :])
                d5 = DP.tile([P, 25, P], f16, tag="d5", name="d5")
                nc.sync.dma_start(d5[:, :, :], wd5_d[b])

                acc3 = accs[b].rearrange("p (a b) -> p a b", a=H)
                attn8 = attn8s[b]
                nc.gpsimd.memset(attn8[:, :, 0:30], 0.0)
                nc.gpsimd.memset(attn8[:, :, 94:128], 0.0)

                def rv5(t, r0, r1):
                    dh, dw = t // 5, t % 5
                    return xcan[:, dh + r0:dh + r1, dw:dw + 64]

                n_pe, n_dve, n_act = K5
                w0sc = wtiles[("w0sc", b)]
                for c in range(2):
                    ps = PP.tile([P, 2048], f32, tag="ps", name="ps")
                    for i in range(n_pe):
                        for j in range(4):
                            r0 = 32 * c + 8 * j
                            nc.tensor.matmul(
                                ps[:, 512 * j:512 * (j + 1)], d5[:, i, :],
                                rv5(i, r0, r0 + 8),
                                start=(i == 0), stop=(i == n_pe - 1))
                    ps3 = ps.rearrange("p (a b) -> p a b", a=32)
                    for h2 in range(2):
                        nc.scalar.activation(
                            acc3[:, 32 * c + 16 * h2:32 * c + 16 * h2 + 16, :],
                            ps3[:, 16 * h2:16 * h2 + 16, :],
                            AF.Identity, bias=wtiles[("b0", b)][:, 0:1],
                            scale=1.0)

                    acc_c = accs[b][:, 2048 * c:2048 * (c + 1)]
                    for i in range(n_dve):
                        t = n_pe + i
                        tmp = SP.tile([P, 2048], f16, tag="ttmp", bufs=3,
                                      name="ttmp")
                        nc.vector.tensor_scalar_mul(
                            tmp.rearrange("p (a b) -> p a b", a=32),
                            rv5(t, 32 * c, 32 * c + 32), w0sc[:, t:t + 1])
                        nc.vector.tensor_tensor(acc_c, tmp[:, :], acc_c,
                                                op=A.add)
                    for i in range(n_act):
                        # ACT lane: multiply on ACT, accumulate on DVE
                        t = n_pe + n_dve + i
                        tmp = SP.tile([P, 2048], f16, tag="ttmp", bufs=3,
                                      name="ttmp")
                        nc.scalar.activation(
                            tmp.rearrange("p (a b) -> p a b", a=32),
                            rv5(t, 32 * c, 32 * c + 32), AF.Identity,
                            bias=0.0, scale=w0sc[:, t:t + 1])
                        nc.vector.tensor_tensor(acc_c, tmp[:, :], acc_c,
                                                op=A.add)
                    # fp8 mirror of this chunk for the branch convs
                    nc.vector.tensor_scalar_mul(
                        attn8[:, 32 * c:32 * c + 32, 30:94],
                        acc3[:, 32 * c:32 * c + 32, :], 1.0)

            # --------------- stage B: dilated branches ---------------------
            def stage_b(b, first):
                attn8 = attn8s[b]
                zts = []
                for nm, k, npair, wpad, zrows, zpad in WBR:
                    zt = CP.tile([P, zrows, 64], f8, tag=f"z{nm}",
                                 name=f"z{nm}")
                    if first:
                        nc.gpsimd.memset(zt[:, 0:zpad, :], 0.0)
                        nc.gpsimd.memset(zt[:, zpad + 64:zrows, :], 0.0)
                    zts.append(zt)
                dsw, dsh = [], []
                for nm, k, npair, wpad, zrows, zpad in WBR:
                    dw_t = BDP.tile([P, npair, 2, P], f8, tag=f"dw{nm}",
                                    name=f"dw{nm}")
                    nc.sync.dma_start(dw_t[:, :, :, :], brd["w" + nm][b])
                    dh_t = BDP.tile([P, npair, 2, P], f8, tag=f"dh{nm}",
                                    name=f"dh{nm}")
                    nc.sync.dma_start(dh_t[:, :, :, :], brd["h" + nm][b])
                    dsw.append(dw_t)
                    dsh.append(dh_t)

                # W-convs: attn8 -> z (fp8 DoubleRow pairs, shift = 3 cols)
                for (nm, k, npair, wpad, zrows, zpad), dw_t, zt in \
                        zip(WBR, dsw, zts):
                    for c in range(2):
                        ps = PP.tile([P, 2048], f32, tag="ps", name="ps")
                        for jp in range(npair):
                            col0 = 30 + 6 * jp - wpad
                            for j in range(4):
                                r0 = 32 * c + 8 * j
                                base = attn8[:, r0:r0 + 8, col0:col0 + 64]
                                rhs = AP(base.tensor, base.offset,
                                         [base.ap[0], [3, 2], [128, 8],
                                          [1, 64]])
                                nc.tensor.matmul(
                                    ps[:, 512 * j:512 * (j + 1)],
                                    dw_t[:, jp, :, :], rhs,
                                    start=(jp == 0), stop=(jp == npair - 1),
                                    perf_mode=DR)
                        ps3 = ps.rearrange("p (a b) -> p a b", a=32)
                        for h2 in range(2):
                            r = zpad + 32 * c + 16 * h2
                            nc.scalar.activation(
                                zt[:, r:r + 16, :],
                                ps3[:, 16 * h2:16 * h2 + 16, :],
                                AF.Identity, bias=0.0, scale=1.0)

                # H-convs: all branches -> one PSUM group (shift = 3 rows),
                # folded into acc with a single STT per chunk (x 1/256)
                ngrp = len(WBR)
                for c in range(2):
                    ps = PP.tile([P, 2048], f32, tag="ps", name="ps")
                    for gi, ((nm, k, npair, wpad, zrows, zpad), dh_t, zt) in \
                            enumerate(zip(WBR, dsh, zts)):
                        for jp in range(npair):
                            row0 = 6 * jp
                            for j in range(4):
                                r0 = 32 * c + 8 * j
                                base = zt[:, row0 + r0:row0 + r0 + 8, :]
                                rhs = AP(base.tensor, base.offset,
                                         [base.ap[0], [192, 2], [64, 8],
                                          [1, 64]])
                                nc.tensor.matmul(
                                    ps[:, 512 * j:512 * (j + 1)],
                                    dh_t[:, jp, :, :], rhs,
                                    start=(gi == 0 and jp == 0),
                                    stop=(gi == ngrp - 1 and
                                          jp == npair - 1),
                                    perf_mode=DR)
                    acc_c = accs[b][:, 2048 * c:2048 * (c + 1)]
                    if os.environ.get("STT", "1") == "1":
                        nc.vector.scalar_tensor_tensor(
                            acc_c, ps[:, :], 1.0 / (BR_SCALE * BR_SCALE),
                            acc_c, op0=A.mult, op1=A.add)
                    else:
                        hs = SP.tile([P, 2048], f16, tag="hs", name="hs")
                        ps3 = ps.rearrange("p (a b) -> p a b", a=32)
                        hs3 = hs.rearrange("p (a b) -> p a b", a=32)
                        for h2 in range(2):
                            nc.scalar.activation(
                                hs3[:, 16 * h2:16 * h2 + 16, :],
                                ps3[:, 16 * h2:16 * h2 + 16, :],
                                AF.Identity, bias=0.0,
                                scale=1.0 / (BR_SCALE * BR_SCALE))
                        nc.vector.tensor_tensor(acc_c, hs[:, :], acc_c,
                                                op=A.add)

            # interleave so stage-B PE work overlaps lane chains
            if os.environ.get("NO_BRANCH", "0") == "1":
                for b in range(NBLK):
                    stage_a(b)
            elif os.environ.get("INTERLEAVE", "1") == "1":
                stage_a(0)
                stage_a(1)
                stage_b(0, True)
                stage_a(2)
                stage_b(1, True)
                stage_a(3)
                stage_b(2, False)
                stage_b(3, False)
            else:
                for b in range(NBLK):
                    stage_a(b)
                for b in range(NBLK):
                    stage_b(b, b < 2)

            # ---- pointwise 1x1 conv + bias + multiply-by-x ----
            for m in range(NBLK):
                sl = slice(m * P, (m + 1) * P)
                xcan = CP.tile([P, 68, 68], f16, tag="xcan", name="xcan")
                nc.sync.dma_start(xcan[:, :, :], xc_d[sl, :, :])
                outb = SP.tile([P, HW], f16, tag="outb", name="outb")
                outb3 = outb.rearrange("p (a b) -> p a b", a=H)
                for nch in range(8):
                    ps = PP.tile([P, 2048], f32, tag="ps", name="ps")
                    for kk in range(NBLK):
                        nc.tensor.matmul(
                            ps[:, 0:512], w3rows[kk][:, m * P:(m + 1) * P],
                            accs[kk][:, nch * 512:(nch + 1) * 512],
                            start=(kk == 0), stop=(kk == NBLK - 1))
                    yb = SP.tile([P, 8, 64], f16, tag="yb", name="yb")
                    nc.scalar.activation(
                        yb[:, :, :],
                        ps[:, 0:512].rearrange("p (a b) -> p a b", a=8),
                        AF.Identity, bias=wtiles[("b3", m)][:, 0:1],
                        scale=1.0)
                    xv = xcan[:, 2 + 8 * nch:2 + 8 * nch + 8, 2:66]
                    nc.vector.tensor_tensor(
                        outb3[:, 8 * nch:8 * nch + 8, :], yb[:, :, :], xv,
                        op=A.mult)
                nc.sync.dma_start(out_d[sl, :, :], outb3[:, :, :])

    if not nc.is_finalized():
        nc.finalize()
    return nc


def _get_nc():
    global _NC
    if _NC is None:
        _NC = _build_nc()
    return _NC


def _prep_inputs(inputs):
    import ml_dtypes
    f8np = ml_dtypes.float8_e4m3

    f = lambda a, shp: np.ascontiguousarray(
        np.asarray(a, dtype=np.float32).reshape(shp))
    g = lambda nm, k: f(inputs[nm], (C, k))

    w0 = g("w0", 25)
    wd5 = np.zeros((NBLK, P, 25, P), np.float16)
    idx = np.arange(P)
    for b in range(NBLK):
        wd5[b, idx, :, idx] = w0[b * P:(b + 1) * P, :].astype(np.float16)

    def pair_stack(w, npair):
        k = w.shape[1]
        d = np.zeros((NBLK, P, npair, 2, P), np.float32)
        for b in range(NBLK):
            wb = w[b * P:(b + 1) * P, :] * BR_SCALE
            for jp in range(npair):
                for i in range(2):
                    t = 2 * jp + i
                    if t < k:
                        d[b, idx, jp, i, idx] = wb[:, t]
        return np.ascontiguousarray(d.astype(f8np))

    # branch-bias vector, folded into the pointwise bias: b3' = b3 + W3@bsumh
    bsumh = (np.asarray(inputs["b0_2"], np.float32)
             + np.asarray(inputs["b1_2"], np.float32)
             + np.asarray(inputs["b2_2"], np.float32)
             + g("w0_2", 7).sum(1) * np.asarray(inputs["b0_1"], np.float32).reshape(C)
             + g("w1_2", 11).sum(1) * np.asarray(inputs["b1_1"], np.float32).reshape(C)
             + g("w2_2", 21).sum(1) * np.asarray(inputs["b2_1"], np.float32).reshape(C))
    w3m = np.asarray(inputs["w3"], np.float32).reshape(C, C)
    b3p = np.asarray(inputs["b3"], np.float32).reshape(C) + w3m @ bsumh

    com = {
        "b0": f(inputs["b0"], (C, 1)),
        "w0sc": w0,
        "wd5": np.ascontiguousarray(wd5),
        "wdw7": pair_stack(g("w0_1", 7), 4),
        "wdh7": pair_stack(g("w0_2", 7), 4),
        "wdw11": pair_stack(g("w1_1", 11), 6),
        "wdh11": pair_stack(g("w1_2", 11), 6),
        "wdw21": pair_stack(g("w2_1", 21), 11),
        "wdh21": pair_stack(g("w2_2", 21), 11),
        "w3": np.ascontiguousarray(w3m.T.astype(np.float16)),
        "b3": np.ascontiguousarray(b3p.reshape(C, 1)),
    }
    x = np.asarray(inputs["x"], np.float32).astype(np.float16)
    xp = np.zeros((x.shape[0], C, 68, 68), np.float16)
    xp[:, :, 2:66, 2:66] = x
    return [dict(com, xcanv=np.ascontiguousarray(xp[i]))
            for i in range(x.shape[0])]


def run(inputs, trace=False):
    from concourse.bass_utils import run_bass_kernel_spmd
    nc = _get_nc()
    in_maps = _prep_inputs(inputs)
    res = run_bass_kernel_spmd(nc, in_maps, core_ids=list(range(len(in_maps))),
                               trace=trace)
    out = np.stack([r["out"] for r in res.results], axis=0).astype(np.float32)
    return out, res


def kernel(**inputs):
    out, _ = run(inputs, trace=False)
    return out
